# revision 8
# baseline (speedup 1.0000x reference)
"""Trainium2 Bass kernel for nn_MultiHeadAttention_51711406244477.

Sharding: 8 cores = (batch b in 0..1) x (sequence chunk j in 0..3).
Each core computes the full module output for its (batch, 512-position chunk):
  - K/V projections over the full sequence (replicated within a batch group)
  - Q projection for its chunk only
  - attention (all 8 heads) for its 512 query positions
  - Wm -> concat -> W1(+folded BN)+ReLU -> W2 epilogue for its chunk
No collectives; host slices/casts inputs per core and reassembles the output.

Precision: bf16 activations/weights on the PE, fp32 PSUM accumulation,
fp32 softmax denominators. Softmax is computed without max-subtraction
(scores are bounded, exp cannot overflow fp32) and masking is applied
multiplicatively after exp, which matches where(mask==0, -1e9) exactly
because masked exp terms underflow to 0 in the reference too.

Host-side foldings:
  - 1/sqrt(dk) into Wq/bq
  - BatchNorm (inference) into W1/b1
  - bv into b1 (via Wm), bm into b1
  - channel permutation to head-major order into Wq/Wk/Wv rows & Wm columns
"""

import numpy as np
import ml_dtypes

import concourse.bacc as bacc
import concourse.bass as bass
import concourse.mybir as mybir
import concourse.tile as tile
from concourse.bass_utils import run_bass_kernel_spmd

BF16 = ml_dtypes.bfloat16
F32 = mybir.dt.float32
BF = mybir.dt.bfloat16

B, D, N, H, DK = 2, 512, 2048, 8, 64
EPS = 1e-5
G = 4          # chunks (cores) per batch
NCH = N // G   # 512 query positions per core
NCORES = 8
MC = N // 128  # 16 key chunks
ROUNDS = [(0, 3), (3, 3), (6, 3), (9, 3), (12, 3), (15, 1)]  # (start chunk, len)

_PROFILE = False          # test.py flips this for NTFF profiling
_LAST_RESULT = [None]     # stash of the last BassKernelResults (for test.py)
_NC_CACHE = [None]


def _build_nc():
    nc = bacc.Bacc("TRN2", target_bir_lowering=False, debug=False,
                   enable_asserts=True, num_devices=NCORES)

    dt_in = [
        ("xq", [D, NCH], BF),          # init_query chunk (natural chan order)
        ("key", [D, N], BF),           # key_t full
        ("val", [D, N], BF),           # value full
        ("maskT", [N, NCH], BF),       # mask[b,0,n0:n0+NCH,:].T  (m, n)
        ("wqT", [D, D], BF),           # (Wq[ch]/8).T
        ("wkT", [D, D], BF),           # Wk[ch].T
        ("wvT", [D, 520], BF),         # Wv aug with zero cols at ones slots
        ("erow", [1, 520], BF),        # 1.0 at ones-column slots
        ("wmT", [D, D], BF),           # Wm[:, ch].T
        ("w1T", [2 * D, 2 * D], BF),   # (s*W1).T
        ("w2T", [2 * D, D], BF),       # W2.T
        ("bq8", [128, 4], F32),
        ("bk2", [128, 4], F32),
        ("b1pp", [128, 8], F32),
        ("b2v", [128, 4], F32),
    ]
    dr = {}
    for name, shape, dt in dt_in:
        dr[name] = nc.dram_tensor(name, shape, dt, kind="ExternalInput")
    out_d = nc.dram_tensor("out", [D, NCH], F32, kind="ExternalOutput")

    AF = mybir.ActivationFunctionType
    OP = mybir.AluOpType

    with tile.TileContext(nc) as tc:
        with tc.tile_pool(name="const", bufs=1) as cpool:
            apool = cpool
            # ---------------- persistent SBUF tensors + input DMAs ----------
            xq_sb = cpool.tile([128, 4, NCH], BF, tag="xq")
            nc.sync.dma_start(xq_sb[:], dr["xq"].ap().rearrange("(t p) n -> p t n", p=128))

            wq_sb = cpool.tile([128, 4, D], BF, tag="wq")
            nc.sync.dma_start(wq_sb[:], dr["wqT"].ap().rearrange("(k p) m -> p k m", p=128))
            wk_sb = cpool.tile([128, 4, D], BF, tag="wk")
            nc.sync.dma_start(wk_sb[:], dr["wkT"].ap().rearrange("(k p) m -> p k m", p=128))
            wv_sb = cpool.tile([128, 4, 520], BF, tag="wv")
            nc.sync.dma_start(wv_sb[:], dr["wvT"].ap().rearrange("(k p) m -> p k m", p=128))
            e_sb = cpool.tile([1, 520], BF, tag="erow")
            nc.sync.dma_start(e_sb[:], dr["erow"].ap())
            wm_sb = cpool.tile([128, 4, D], BF, tag="wm")
            nc.sync.dma_start(wm_sb[:], dr["wmT"].ap().rearrange("(k p) m -> p k m", p=128))
            w1_sb = cpool.tile([128, 8, 2 * D], BF, tag="w1")
            nc.sync.dma_start(w1_sb[:], dr["w1T"].ap().rearrange("(k p) m -> p k m", p=128))
            w2_sb = cpool.tile([128, 8, D], BF, tag="w2")
            nc.sync.dma_start(w2_sb[:], dr["w2T"].ap().rearrange("(k p) m -> p k m", p=128))

            bq_sb = cpool.tile([128, 4], F32, tag="bq")
            nc.sync.dma_start(bq_sb[:], dr["bq8"].ap())
            bk_sb = cpool.tile([128, 4], F32, tag="bk")
            nc.sync.dma_start(bk_sb[:], dr["bk2"].ap())
            b1_sb = cpool.tile([128, 8], F32, tag="b1")
            nc.sync.dma_start(b1_sb[:], dr["b1pp"].ap())
            b2_sb = cpool.tile([128, 4], F32, tag="b2")
            nc.sync.dma_start(b2_sb[:], dr["b2v"].ap())

            ones_bf = cpool.tile([1, 128], BF, tag="ones_bf")
            nc.vector.memset(ones_bf[:], 1.0)
            ones_f = cpool.tile([1, 64], F32, tag="ones_f")
            nc.vector.memset(ones_f[:], 1.0)

            # dummy exp: pulls the ACT table load to kernel start
            dumm = cpool.tile([1, 16], F32, tag="dumm")
            nc.vector.memset(dumm[:], 0.0)
            dumo = cpool.tile([1, 16], BF, tag="dumo")
            nc.scalar.activation(dumo[:], dumm[:], AF.Exp)

            q_sb = apool.tile([128, 4, NCH], BF, tag="q")
            k_sb = apool.tile([128, 4, N], BF, tag="k")
            vt_sb = apool.tile([128, MC, 520], BF, tag="vt")
            xfull_sb = apool.tile([128, 4, NCH], BF, tag="xfull")
            y_sb = apool.tile([128, 4, NCH], BF, tag="y")
            h_sb = apool.tile([128, 8, NCH], BF, tag="h")
            out_sb = apool.tile([128, 4, NCH], F32, tag="outsb")

            # ---------------- projections ----------------------------------
            with (
                tc.tile_pool(name="kvin", bufs=1) as kvp,
                tc.tile_pool(name="pproj", bufs=4, space=bass.MemorySpace.PSUM) as pj,
            ):
                key_sb = kvp.tile([128, 4, N], BF, tag="key")
                nc.sync.dma_start(key_sb[:],
                                  dr["key"].ap().rearrange("(t p) n -> p t n", p=128))
                val_sb = kvp.tile([128, 4, MC, 128], BF, tag="val")
                nc.sync.dma_start(
                    val_sb[:],
                    dr["val"].ap().rearrange("(t p) (c q) -> p t c q", p=128, q=128))
                # V^T projection: vt[m, d'] with ones columns (520 = 4 pairs x 130)
                for c in range(MC):
                    for hf in range(2):
                        ps = pj.tile([128, 260], F32, tag="pv")
                        sl = slice(260 * hf, 260 * hf + 260)
                        for kk in range(4):
                            nc.tensor.matmul(ps[:], val_sb[:, kk, c, :], wv_sb[:, kk, sl],
                                             start=(kk == 0), stop=False)
                        nc.tensor.matmul(ps[:], ones_bf[0:1, :], e_sb[0:1, sl],
                                         start=False, stop=True)
                        nc.any.tensor_copy(vt_sb[:, c, sl], ps[:])
                # K projection (full sequence)
                for mo in range(4):
                    for mt in range(4):
                        ps = pj.tile([128, 512], F32, tag="pk")
                        for kk in range(4):
                            nc.tensor.matmul(
                                ps[:], wk_sb[:, kk, 128 * mo:128 * mo + 128],
                                key_sb[:, kk, 512 * mt:512 * mt + 512],
                                start=(kk == 0), stop=(kk == 3))
                        nc.any.tensor_scalar_add(
                            k_sb[:, mo, 512 * mt:512 * mt + 512], ps[:], bk_sb[:, mo:mo + 1])
                # Q projection (chunk only; 1/8 folded into weights/bias)
                for mo in range(4):
                    ps = pj.tile([128, NCH], F32, tag="pk")
                    for kk in range(4):
                        nc.tensor.matmul(ps[:], wq_sb[:, kk, 128 * mo:128 * mo + 128],
                                         xq_sb[:, kk, :], start=(kk == 0), stop=(kk == 3))
                    nc.any.tensor_scalar_add(q_sb[:, mo, :], ps[:], bq_sb[:, mo:mo + 1])

            # ---------------- attention ------------------------------------
            with tc.tile_pool(name="attx", bufs=1) as axp:
                xun_sb = axp.tile([65, 8, NCH], F32, tag="xun")
                with (
                    tc.tile_pool(name="attm", bufs=1) as amp,
                    tc.tile_pool(name="ptiles", bufs=2) as ppool,
                    tc.tile_pool(name="patt", bufs=2, space=bass.MemorySpace.PSUM) as pa,
                ):
                    msk_sb = amp.tile([128, MC, NCH], BF, tag="mask")
                    nc.sync.dma_start(
                        msk_sb[:], dr["maskT"].ap().rearrange("(c p) n -> p c n", p=128))
                    for p in range(4):  # head pair (2p, 2p+1)
                        pvA = pa.tile([65, NCH], F32, tag="pv")
                        pvB = pa.tile([65, NCH], F32, tag="pv")
                        for (c0, ln) in ROUNDS:
                            scA = pa.tile([128, 3, NCH], F32, tag="sc")
                            scB = pa.tile([128, 3, NCH], F32, tag="sc")
                            for ci in range(ln):
                                c = c0 + ci
                                nc.tensor.matmul(scA[:, ci, :],
                                                 k_sb[0:64, p, 128 * c:128 * c + 128],
                                                 q_sb[0:64, p, :], start=True, stop=True)
                                nc.tensor.matmul(scB[:, ci, :],
                                                 k_sb[64:128, p, 128 * c:128 * c + 128],
                                                 q_sb[64:128, p, :], start=True, stop=True)
                            pA = ppool.tile([128, 3, NCH], BF, tag="pa")
                            pB = ppool.tile([128, 3, NCH], BF, tag="pb")
                            nc.scalar.activation(pA[:, 0:ln, :], scA[:, 0:ln, :], AF.Exp)
                            nc.scalar.activation(pB[:, 0:ln, :], scB[:, 0:ln, :], AF.Exp)
                            pmA = ppool.tile([128, 3, NCH], BF, tag="pma")
                            pmB = ppool.tile([128, 3, NCH], BF, tag="pmb")
                            nc.vector.tensor_tensor(pmA[:, 0:ln, :], pA[:, 0:ln, :],
                                                    msk_sb[:, c0:c0 + ln, :], OP.mult)
                            nc.gpsimd.tensor_tensor(pmB[:, 0:ln, :], pB[:, 0:ln, :],
                                                    msk_sb[:, c0:c0 + ln, :], OP.mult)
                            for ci in range(ln):
                                c = c0 + ci
                                nc.tensor.matmul(pvA[:],
                                                 vt_sb[:, c, 130 * p:130 * p + 65],
                                                 pmA[:, ci, :],
                                                 start=(c == 0), stop=(c == MC - 1),
                                                 skip_group_check=True)
                                nc.tensor.matmul(pvB[:],
                                                 vt_sb[:, c, 130 * p + 65:130 * p + 130],
                                                 pmB[:, ci, :],
                                                 start=(c == 0), stop=(c == MC - 1),
                                                 skip_group_check=True)
                        nc.any.tensor_copy(xun_sb[:, 2 * p, :], pvA[:])
                        nc.any.tensor_copy(xun_sb[:, 2 * p + 1, :], pvB[:])

                # ------------- softmax normalization -----------------------
                with (
                    tc.tile_pool(name="nrm", bufs=1) as nrp,
                    tc.tile_pool(name="pep1", bufs=4, space=bass.MemorySpace.PSUM) as pe1,
                ):
                    den_sb = nrp.tile([8, NCH], F32, tag="den")
                    lnd_sb = nrp.tile([8, NCH], F32, tag="lnd")
                    rec_sb = nrp.tile([8, NCH], F32, tag="rec")
                    rec1_sb = nrp.tile([1, 8, NCH], F32, tag="rec1")
                    xnorm_sb = nrp.tile([64, 8, NCH], BF, tag="xnorm")
                    # denominators live in xun row 64; shift to partitions 0..7
                    nc.sync.dma_start(den_sb[:], xun_sb[64:65, :, :])
                    # rec = exp(-ln(den)) : stays in the natural_log_exp table set
                    nc.scalar.activation(lnd_sb[:], den_sb[:], AF.Ln)
                    nc.scalar.activation(rec_sb[:], lnd_sb[:], AF.Exp, scale=-1.0)
                    # matmul rhs must sit at base partition 0: [8,n] -> [1,8,n]
                    nc.sync.dma_start(rec1_sb[:], rec_sb[:])
                    for hh in range(8):
                        bc = pe1.tile([64, NCH], F32, tag="bc")
                        nc.tensor.matmul(bc[:], ones_f[0:1, :], rec1_sb[0:1, hh, :],
                                         start=True, stop=True)
                        nc.any.tensor_tensor(xnorm_sb[:, hh, :], xun_sb[0:64, hh, :],
                                             bc[:], OP.mult)
                    # partition shift: even heads -> partitions 0-63; odd -> 64-127
                    nc.sync.dma_start(xfull_sb[0:64, :, :], xnorm_sb[:, 0:8:2, :])
                    nc.sync.dma_start(xfull_sb[64:128, :, :], xnorm_sb[:, 1:8:2, :])

            # ---------------- epilogue -------------------------------------
            with tc.tile_pool(name="pep2", bufs=4, space=bass.MemorySpace.PSUM) as pe:
                for mo in range(4):
                    ps = pe.tile([128, NCH], F32, tag="mm")
                    for kk in range(4):
                        nc.tensor.matmul(ps[:], wm_sb[:, kk, 128 * mo:128 * mo + 128],
                                         xfull_sb[:, kk, :], start=(kk == 0), stop=(kk == 3))
                    nc.any.tensor_copy(y_sb[:, mo, :], ps[:])
                for mo in range(8):
                    ps = pe.tile([128, NCH], F32, tag="mm")
                    for kk in range(8):
                        rhs = y_sb[:, kk, :] if kk < 4 else xq_sb[:, kk - 4, :]
                        nc.tensor.matmul(ps[:], w1_sb[:, kk, 128 * mo:128 * mo + 128],
                                         rhs, start=(kk == 0), stop=(kk == 7))
                    # relu(x + b): (ps + b1pp) max 0
                    nc.any.tensor_scalar(h_sb[:, mo, :], ps[:], b1_sb[:, mo:mo + 1], 0.0,
                                         OP.add, OP.max)
                for mo in range(4):
                    ps = pe.tile([128, NCH], F32, tag="mm")
                    for kk in range(8):
                        nc.tensor.matmul(ps[:], w2_sb[:, kk, 128 * mo:128 * mo + 128],
                                         h_sb[:, kk, :], start=(kk == 0), stop=(kk == 7))
                    nc.any.tensor_scalar_add(out_sb[:, mo, :], ps[:], b2_sb[:, mo:mo + 1])
                nc.sync.dma_start(out_d.ap().rearrange("(t p) n -> p t n", p=128), out_sb[:])

    nc.compile()
    return nc


def _prep_inputs(inputs):
    """Host-side slicing / casting / weight folding -> per-core input maps."""
    f = {k: np.asarray(v, np.float32) if np.asarray(v).dtype != np.int32
         else np.asarray(v) for k, v in inputs.items()}
    ch = np.array([dk * H + h for h in range(H) for dk in range(DK)])

    s = f["gamma"] / np.sqrt(f["rvar"] + EPS)
    W1p = s[:, None] * f["W1"]
    bmp = f["Wm"] @ f["bv"] + f["bm"]
    b1pp = W1p[:, :D] @ bmp + s * (f["b1"] - f["rmean"]) + f["beta"]

    wqT = np.ascontiguousarray((f["Wq"][ch] / 8).T).astype(BF16)
    wkT = np.ascontiguousarray(f["Wk"][ch].T).astype(BF16)
    wvT = np.zeros((D, 520), np.float32)
    for p in range(4):
        wvT[:, 130 * p:130 * p + 64] = f["Wv"][ch[128 * p:128 * p + 64]].T
        wvT[:, 130 * p + 65:130 * p + 129] = f["Wv"][ch[128 * p + 64:128 * p + 128]].T
    erow = np.zeros((1, 520), np.float32)
    for p in range(4):
        erow[0, 130 * p + 64] = 1.0
        erow[0, 130 * p + 129] = 1.0
    wmT = np.ascontiguousarray(f["Wm"][:, ch].T).astype(BF16)
    w1T = np.ascontiguousarray(W1p.T).astype(BF16)
    w2T = np.ascontiguousarray(f["W2"].T).astype(BF16)

    shared = {
        "wqT": wqT, "wkT": wkT, "wvT": wvT.astype(BF16), "erow": erow.astype(BF16),
        "wmT": wmT, "w1T": w1T, "w2T": w2T,
        "bq8": np.ascontiguousarray((f["bq"][ch] / 8).reshape(4, 128).T).astype(np.float32),
        "bk2": np.ascontiguousarray(f["bk"][ch].reshape(4, 128).T).astype(np.float32),
        "b1pp": np.ascontiguousarray(b1pp.reshape(8, 128).T).astype(np.float32),
        "b2v": np.ascontiguousarray(f["b2"].reshape(4, 128).T).astype(np.float32),
    }
    in_maps = []
    for c in range(NCORES):
        b, j = divmod(c, G)
        n0 = j * NCH
        m = dict(shared)
        m["xq"] = np.ascontiguousarray(f["init_query"][b][:, n0:n0 + NCH]).astype(BF16)
        m["key"] = f["key_t"][b].astype(BF16)
        m["val"] = f["value"][b].astype(BF16)
        m["maskT"] = np.ascontiguousarray(f["mask"][b, 0, n0:n0 + NCH, :].T).astype(BF16)
        in_maps.append(m)
    return in_maps


def kernel(**inputs) -> np.ndarray:
    if _NC_CACHE[0] is None:
        _NC_CACHE[0] = _build_nc()
    nc = _NC_CACHE[0]
    in_maps = _prep_inputs(inputs)
    res = run_bass_kernel_spmd(nc, in_maps, list(range(NCORES)), trace=_PROFILE)
    _LAST_RESULT[0] = res
    out = np.zeros((B, D, N), np.float32)
    for c in range(NCORES):
        b, j = divmod(c, G)
        out[b, :, j * NCH:(j + 1) * NCH] = res.results[c]["out"]
    return out


# revision 15
# speedup vs baseline: 1.0830x; 1.0830x over previous
"""Trainium2 Bass kernel for nn_MultiHeadAttention_51711406244477.

Sharding: 8 cores = (batch b in 0..1) x (sequence chunk j in 0..3).
Each core computes the full module output for its (batch, 512-position chunk):
  - K/V projections over the full sequence (replicated within a batch group)
  - Q projection for its chunk only
  - attention (all 8 heads) for its 512 query positions
  - Wm -> concat -> W1(+folded BN)+ReLU -> W2 epilogue for its chunk
No collectives; host slices/casts inputs per core and reassembles the output.

Precision: bf16 activations/weights on the PE, fp32 PSUM accumulation,
fp32 softmax denominators. Softmax is computed without max-subtraction
(scores are bounded, exp cannot overflow fp32) and masking is applied
multiplicatively after exp, which matches where(mask==0, -1e9) exactly
because masked exp terms underflow to 0 in the reference too.

Host-side foldings:
  - 1/sqrt(dk) into Wq/bq
  - BatchNorm (inference) into W1/b1
  - bv into b1 (via Wm), bm into b1
  - channel permutation to head-major order into Wq/Wk/Wv rows & Wm columns
"""

import numpy as np
import ml_dtypes

import concourse.bacc as bacc
import concourse.bass as bass
import concourse.mybir as mybir
import concourse.tile as tile
from concourse.bass_utils import run_bass_kernel_spmd

BF16 = ml_dtypes.bfloat16
F32 = mybir.dt.float32
BF = mybir.dt.bfloat16

B, D, N, H, DK = 2, 512, 2048, 8, 64
EPS = 1e-5
G = 4          # chunks (cores) per batch
NCH = N // G   # 512 query positions per core
NCORES = 8
MC = N // 128  # 16 key chunks
ROUNDS = [(2 * i, 2) for i in range(8)]  # (start chunk, len)

_PROFILE = False          # test.py flips this for NTFF profiling
_LAST_RESULT = [None]     # stash of the last BassKernelResults (for test.py)
_NC_CACHE = [None]


def _build_nc():
    nc = bacc.Bacc("TRN2", target_bir_lowering=False, debug=False,
                   enable_asserts=True, num_devices=NCORES)

    dt_in = [
        ("xq", [D, NCH], BF),          # init_query chunk (natural chan order)
        ("key", [D, N], BF),           # key_t full
        ("val", [D, N], BF),           # value full
        ("maskT", [N, NCH], BF),       # mask[b,0,n0:n0+NCH,:].T  (m, n)
        ("wqT", [D, D], BF),           # (Wq[ch]/8).T
        ("wkT", [D, D], BF),           # Wk[ch].T
        ("wvT", [D, 520], BF),         # Wv aug with zero cols at ones slots
        ("wmT", [D, D], BF),           # Wm[:, ch].T
        ("w1T", [2 * D, 2 * D], BF),   # (s*W1).T
        ("w2T", [2 * D, D], BF),       # W2.T
        ("bq8", [128, 4], F32),
        ("bk2", [128, 4], F32),
        ("b1pp", [128, 8], F32),
        ("b2v", [128, 4], F32),
    ]
    dr = {}
    for name, shape, dt in dt_in:
        dr[name] = nc.dram_tensor(name, shape, dt, kind="ExternalInput")
    out_d = nc.dram_tensor("out", [D, NCH], F32, kind="ExternalOutput")

    AF = mybir.ActivationFunctionType
    OP = mybir.AluOpType

    with tile.TileContext(nc) as tc:
        with tc.tile_pool(name="const", bufs=1) as cpool:
            # ---------- persistent SBUF tensors; DMAs in consumption order --
            wk_sb = cpool.tile([128, 4, D], BF, tag="wk")
            nc.sync.dma_start(wk_sb[:], dr["wkT"].ap().rearrange("(k p) m -> p k m", p=128))
            key_sb = cpool.tile([128, 4, N], BF, tag="key")
            nc.sync.dma_start(key_sb[:], dr["key"].ap().rearrange("(t p) n -> p t n", p=128))
            wq_sb = cpool.tile([128, 4, D], BF, tag="wq")
            nc.sync.dma_start(wq_sb[:], dr["wqT"].ap().rearrange("(k p) m -> p k m", p=128))
            xq_sb = cpool.tile([128, 4, NCH], BF, tag="xq")
            nc.sync.dma_start(xq_sb[:], dr["xq"].ap().rearrange("(t p) n -> p t n", p=128))
            wv_sb = cpool.tile([128, 4, 520], BF, tag="wv")
            nc.sync.dma_start(wv_sb[:], dr["wvT"].ap().rearrange("(k p) m -> p k m", p=128))
            val_sb = cpool.tile([128, 4, MC, 128], BF, tag="val")
            nc.sync.dma_start(
                val_sb[:],
                dr["val"].ap().rearrange("(t p) (c q) -> p t c q", p=128, q=128))
            bq_sb = cpool.tile([128, 4], F32, tag="bq")
            nc.sync.dma_start(bq_sb[:], dr["bq8"].ap())
            bk_sb = cpool.tile([128, 4], F32, tag="bk")
            nc.sync.dma_start(bk_sb[:], dr["bk2"].ap())
            msk_sb = cpool.tile([128, MC, NCH], BF, tag="mask")
            nc.sync.dma_start(msk_sb[:], dr["maskT"].ap().rearrange("(c p) n -> p c n", p=128))
            b1_sb = cpool.tile([128, 8], F32, tag="b1")
            nc.sync.dma_start(b1_sb[:], dr["b1pp"].ap())
            b2_sb = cpool.tile([128, 4], F32, tag="b2")
            nc.sync.dma_start(b2_sb[:], dr["b2v"].ap())
            # epilogue weights: DMAs emitted late (only needed at ~2/3 of kernel)
            wm_sb = cpool.tile([128, 4, D], BF, tag="wm")
            w1_sb = cpool.tile([128, 8, 2 * D], BF, tag="w1")
            w2_sb = cpool.tile([128, 8, D], BF, tag="w2")

            ones_f = cpool.tile([1, 64], F32, tag="ones_f")
            nc.vector.memset(ones_f[:], 1.0)

            # dummy exp: pulls the ACT table load to kernel start
            dumm = cpool.tile([1, 16], F32, tag="dumm")
            nc.vector.memset(dumm[:], 0.0)
            dumo = cpool.tile([1, 16], BF, tag="dumo")
            nc.scalar.activation(dumo[:], dumm[:], AF.Exp)

            q_sb = cpool.tile([128, 4, NCH], BF, tag="q")
            k_sb = cpool.tile([128, 4, N], BF, tag="k")
            vt_sb = cpool.tile([128, MC, 520], BF, tag="vt")
            xfull_sb = cpool.tile([128, 4, NCH], BF, tag="xfull")
            y_sb = cpool.tile([128, 4, NCH], BF, tag="y")
            h_sb = cpool.tile([128, 8, NCH], BF, tag="h")
            out_sb = cpool.tile([128, 4, NCH], F32, tag="outsb")

            # ---------------- projections ----------------------------------
            with tc.tile_pool(name="pproj", bufs=4, space=bass.MemorySpace.PSUM) as pj:
                # K projection (full sequence)
                for mo in range(4):
                    for mt in range(4):
                        ps = pj.tile([128, 512], F32, tag="pk")
                        for kk in range(4):
                            nc.tensor.matmul(
                                ps[:], wk_sb[:, kk, 128 * mo:128 * mo + 128],
                                key_sb[:, kk, 512 * mt:512 * mt + 512],
                                start=(kk == 0), stop=(kk == 3))
                        nc.any.tensor_scalar_add(
                            k_sb[:, mo, 512 * mt:512 * mt + 512], ps[:], bk_sb[:, mo:mo + 1])
                # Q projection (chunk only; 1/8 folded into weights/bias)
                for mo in range(4):
                    ps = pj.tile([128, NCH], F32, tag="pk")
                    for kk in range(4):
                        nc.tensor.matmul(ps[:], wq_sb[:, kk, 128 * mo:128 * mo + 128],
                                         xq_sb[:, kk, :], start=(kk == 0), stop=(kk == 3))
                    nc.any.tensor_scalar_add(q_sb[:, mo, :], ps[:], bq_sb[:, mo:mo + 1])
                # V^T projection: vt[m, d'] (520 = 4 pairs x 130, ones cols memset)
                for c in range(MC):
                    for hf in range(2):
                        ps = pj.tile([128, 260], F32, tag="pv")
                        sl = slice(260 * hf, 260 * hf + 260)
                        for kk in range(4):
                            nc.tensor.matmul(ps[:], val_sb[:, kk, c, :], wv_sb[:, kk, sl],
                                             start=(kk == 0), stop=(kk == 3))
                        nc.any.tensor_copy(vt_sb[:, c, sl], ps[:])
                # ones columns at 64 + 65k (k=0..7) in each chunk block
                nc.vector.memset(vt_sb[:, :, 64:520:65], 1.0)

            # ---------------- attention ------------------------------------
            with tc.tile_pool(name="attx", bufs=1) as axp:
                xun_sb = axp.tile([65, 8, NCH], F32, tag="xun")
                rec1_sb = axp.tile([1, 8, NCH], F32, tag="rec1")
                with (
                    tc.tile_pool(name="ptiles", bufs=2) as ppool,
                    tc.tile_pool(name="patt", bufs=2, space=bass.MemorySpace.PSUM) as pa,
                ):
                    for p in range(4):  # head pair (2p, 2p+1)
                        pvA = pa.tile([65, NCH], F32, tag="pva")
                        pvB = pa.tile([65, NCH], F32, tag="pvb")
                        for ri, (c0, ln) in enumerate(ROUNDS):
                            scA = pa.tile([128, 2, NCH], F32, tag="sc")
                            scB = pa.tile([128, 2, NCH], F32, tag="sc")
                            for ci in range(ln):
                                c = c0 + ci
                                nc.tensor.matmul(scA[:, ci, :],
                                                 k_sb[0:64, p, 128 * c:128 * c + 128],
                                                 q_sb[0:64, p, :], start=True, stop=True)
                                nc.tensor.matmul(scB[:, ci, :],
                                                 k_sb[64:128, p, 128 * c:128 * c + 128],
                                                 q_sb[64:128, p, :], start=True, stop=True)
                            pA = ppool.tile([128, 2, NCH], BF, tag="pa")
                            pB = ppool.tile([128, 2, NCH], BF, tag="pb")
                            nc.scalar.activation(pA[:, 0:ln, :], scA[:, 0:ln, :], AF.Exp)
                            nc.scalar.activation(pB[:, 0:ln, :], scB[:, 0:ln, :], AF.Exp)
                            pmA = ppool.tile([128, 2, NCH], BF, tag="pma")
                            pmB = ppool.tile([128, 2, NCH], BF, tag="pmb")
                            nc.vector.tensor_tensor(pmA[:, 0:ln, :], pA[:, 0:ln, :],
                                                    msk_sb[:, c0:c0 + ln, :], OP.mult)
                            # balance the mask multiplies: gpsimd is ~2x slower
                            engB = nc.gpsimd if (ri % 2 == 0) else nc.vector
                            engB.tensor_tensor(pmB[:, 0:ln, :], pB[:, 0:ln, :],
                                               msk_sb[:, c0:c0 + ln, :], OP.mult)
                            for ci in range(ln):
                                c = c0 + ci
                                nc.tensor.matmul(pvA[:],
                                                 vt_sb[:, c, 130 * p:130 * p + 65],
                                                 pmA[:, ci, :],
                                                 start=(c == 0), stop=(c == MC - 1),
                                                 skip_group_check=True)
                                nc.tensor.matmul(pvB[:],
                                                 vt_sb[:, c, 130 * p + 65:130 * p + 130],
                                                 pmB[:, ci, :],
                                                 start=(c == 0), stop=(c == MC - 1),
                                                 skip_group_check=True)
                        nc.any.tensor_copy(xun_sb[:, 2 * p, :], pvA[:])
                        nc.any.tensor_copy(xun_sb[:, 2 * p + 1, :], pvB[:])
                        # per-pair denominator pipeline (overlaps next pair)
                        den_p = ppool.tile([2, NCH], F32, tag="denp", name=f"den{p}", bufs=1)
                        nc.sync.dma_start(den_p[:], xun_sb[64:65, 2 * p:2 * p + 2, :])
                        lnd_p = ppool.tile([2, NCH], F32, tag="lndp", name=f"lnd{p}", bufs=1)
                        nc.scalar.activation(lnd_p[:], den_p[:], AF.Ln)
                        # rec = exp(-ln(den)): stays in the natural_log_exp set
                        rec_p = ppool.tile([2, NCH], F32, tag="recp", name=f"rec{p}", bufs=1)
                        nc.scalar.activation(rec_p[:], lnd_p[:], AF.Exp, scale=-1.0)
                        nc.sync.dma_start(rec1_sb[0:1, 2 * p:2 * p + 2, :], rec_p[:])

                # late epilogue-weight DMAs (overlap attention)
                nc.sync.dma_start(wm_sb[:], dr["wmT"].ap().rearrange("(k p) m -> p k m", p=128))
                nc.sync.dma_start(w1_sb[:], dr["w1T"].ap().rearrange("(k p) m -> p k m", p=128))
                nc.sync.dma_start(w2_sb[:], dr["w2T"].ap().rearrange("(k p) m -> p k m", p=128))

                # ------------- normalization + epilogue --------------------
                with (
                    tc.tile_pool(name="nrm", bufs=1) as nrp,
                    tc.tile_pool(name="pep", bufs=1, space=bass.MemorySpace.PSUM) as pep,
                ):
                    xnorm_sb = nrp.tile([64, 8, NCH], BF, tag="xnorm")

                    # W1-x preheat: x-half of W1 runs while normalization drains
                    hx = []
                    for mo in range(6):
                        ps = pep.tile([128, NCH], F32, tag=f"hx{mo}", name=f"hxt{mo}")
                        hx.append(ps)
                        for kk in range(4, 8):
                            nc.tensor.matmul(ps[:], w1_sb[:, kk, 128 * mo:128 * mo + 128],
                                             xq_sb[:, kk - 4, :], start=(kk == 4),
                                             stop=False, skip_group_check=True)

                    bcs = [pep.tile([64, NCH], F32, tag=f"bc{i}", name=f"bct{i}")
                           for i in range(2)]
                    for hh in range(8):
                        bc = bcs[hh % 2]
                        nc.tensor.matmul(bc[:], ones_f[0:1, :], rec1_sb[0:1, hh, :],
                                         start=True, stop=True)
                        nc.any.tensor_tensor(xnorm_sb[:, hh, :], xun_sb[0:64, hh, :],
                                             bc[:], OP.mult)
                    # partition shift: even heads -> partitions 0-63; odd -> 64-127
                    nc.sync.dma_start(xfull_sb[0:64, :, :], xnorm_sb[:, 0:8:2, :])
                    nc.sync.dma_start(xfull_sb[64:128, :, :], xnorm_sb[:, 1:8:2, :])

                    # Wm (uses the 2 bc slots' banks via separate tags is fine:
                    # bc tiles are released after the last tensor_tensor)
                    for mo in range(4):
                        ps = pep.tile([128, NCH], F32, tag=f"bc{mo % 2}")
                        for kk in range(4):
                            nc.tensor.matmul(ps[:], wm_sb[:, kk, 128 * mo:128 * mo + 128],
                                             xfull_sb[:, kk, :], start=(kk == 0),
                                             stop=(kk == 3))
                        nc.any.tensor_copy(y_sb[:, mo, :], ps[:])
                    # W1 y-half accumulates onto the preheated psum tiles
                    for mo in range(8):
                        if mo < 6:
                            ps = hx[mo]
                            for kk in range(4):
                                nc.tensor.matmul(ps[:], w1_sb[:, kk, 128 * mo:128 * mo + 128],
                                                 y_sb[:, kk, :], start=False,
                                                 stop=(kk == 3), skip_group_check=True)
                        else:
                            ps = pep.tile([128, NCH], F32, tag=f"bc{mo % 2}")
                            for kk in range(8):
                                rhs = y_sb[:, kk, :] if kk < 4 else xq_sb[:, kk - 4, :]
                                nc.tensor.matmul(ps[:], w1_sb[:, kk, 128 * mo:128 * mo + 128],
                                                 rhs, start=(kk == 0), stop=(kk == 7))
                        # relu(x + b): (ps + b1pp) max 0
                        nc.any.tensor_scalar(h_sb[:, mo, :], ps[:], b1_sb[:, mo:mo + 1],
                                             0.0, OP.add, OP.max)
                    for mo in range(4):
                        ps = pep.tile([128, NCH], F32, tag=f"hx{mo}")
                        for kk in range(8):
                            nc.tensor.matmul(ps[:], w2_sb[:, kk, 128 * mo:128 * mo + 128],
                                             h_sb[:, kk, :], start=(kk == 0), stop=(kk == 7))
                        nc.any.tensor_scalar_add(out_sb[:, mo, :], ps[:], b2_sb[:, mo:mo + 1])
                        nc.sync.dma_start(
                            out_d.ap().rearrange("(t p) n -> p t n", p=128)[:, mo, :],
                            out_sb[:, mo, :])

    nc.compile()
    return nc


def _prep_inputs(inputs):
    """Host-side slicing / casting / weight folding -> per-core input maps."""
    f = {k: np.asarray(v, np.float32) if np.asarray(v).dtype != np.int32
         else np.asarray(v) for k, v in inputs.items()}
    ch = np.array([dk * H + h for h in range(H) for dk in range(DK)])

    s = f["gamma"] / np.sqrt(f["rvar"] + EPS)
    W1p = s[:, None] * f["W1"]
    bmp = f["Wm"] @ f["bv"] + f["bm"]
    b1pp = W1p[:, :D] @ bmp + s * (f["b1"] - f["rmean"]) + f["beta"]

    wqT = np.ascontiguousarray((f["Wq"][ch] / 8).T).astype(BF16)
    wkT = np.ascontiguousarray(f["Wk"][ch].T).astype(BF16)
    wvT = np.zeros((D, 520), np.float32)
    for p in range(4):
        wvT[:, 130 * p:130 * p + 64] = f["Wv"][ch[128 * p:128 * p + 64]].T
        wvT[:, 130 * p + 65:130 * p + 129] = f["Wv"][ch[128 * p + 64:128 * p + 128]].T
    wmT = np.ascontiguousarray(f["Wm"][:, ch].T).astype(BF16)
    w1T = np.ascontiguousarray(W1p.T).astype(BF16)
    w2T = np.ascontiguousarray(f["W2"].T).astype(BF16)

    shared = {
        "wqT": wqT, "wkT": wkT, "wvT": wvT.astype(BF16),
        "wmT": wmT, "w1T": w1T, "w2T": w2T,
        "bq8": np.ascontiguousarray((f["bq"][ch] / 8).reshape(4, 128).T).astype(np.float32),
        "bk2": np.ascontiguousarray(f["bk"][ch].reshape(4, 128).T).astype(np.float32),
        "b1pp": np.ascontiguousarray(b1pp.reshape(8, 128).T).astype(np.float32),
        "b2v": np.ascontiguousarray(f["b2"].reshape(4, 128).T).astype(np.float32),
    }
    in_maps = []
    for c in range(NCORES):
        b, j = divmod(c, G)
        n0 = j * NCH
        m = dict(shared)
        m["xq"] = np.ascontiguousarray(f["init_query"][b][:, n0:n0 + NCH]).astype(BF16)
        m["key"] = f["key_t"][b].astype(BF16)
        m["val"] = f["value"][b].astype(BF16)
        m["maskT"] = np.ascontiguousarray(f["mask"][b, 0, n0:n0 + NCH, :].T).astype(BF16)
        in_maps.append(m)
    return in_maps


def kernel(**inputs) -> np.ndarray:
    if _NC_CACHE[0] is None:
        _NC_CACHE[0] = _build_nc()
    nc = _NC_CACHE[0]
    in_maps = _prep_inputs(inputs)
    res = run_bass_kernel_spmd(nc, in_maps, list(range(NCORES)), trace=_PROFILE)
    _LAST_RESULT[0] = res
    out = np.zeros((B, D, N), np.float32)
    for c in range(NCORES):
        b, j = divmod(c, G)
        out[b, :, j * NCH:(j + 1) * NCH] = res.results[c]["out"]
    return out


# revision 18
# speedup vs baseline: 1.0973x; 1.0131x over previous
"""Trainium2 Bass kernel for nn_MultiHeadAttention_51711406244477.

Sharding: 8 cores = (batch b in 0..1) x (sequence chunk j in 0..3).
Each core computes the full module output for its (batch, 512-position chunk):
  - K/V projections over the full sequence (replicated within a batch group)
  - Q projection for its chunk only
  - attention (all 8 heads) for its 512 query positions
  - Wm -> concat -> W1(+folded BN)+ReLU -> W2 epilogue for its chunk
No collectives; host slices/casts inputs per core and reassembles the output.

Precision: bf16 activations/weights on the PE, fp32 PSUM accumulation,
fp32 softmax denominators. Softmax is computed without max-subtraction
(scores are bounded, exp cannot overflow fp32) and masking is applied
multiplicatively after exp, which matches where(mask==0, -1e9) exactly
because masked exp terms underflow to 0 in the reference too.

Host-side foldings:
  - 1/sqrt(dk) into Wq/bq
  - BatchNorm (inference) into W1/b1
  - bv into b1 (via Wm), bm into b1
  - channel permutation to head-major order into Wq/Wk/Wv rows & Wm columns
"""

import numpy as np
import ml_dtypes

import concourse.bacc as bacc
import concourse.bass as bass
import concourse.mybir as mybir
import concourse.tile as tile
from concourse.bass_utils import run_bass_kernel_spmd

BF16 = ml_dtypes.bfloat16
F32 = mybir.dt.float32
BF = mybir.dt.bfloat16

B, D, N, H, DK = 2, 512, 2048, 8, 64
EPS = 1e-5
G = 4          # chunks (cores) per batch
NCH = N // G   # 512 query positions per core
NCORES = 8
MC = N // 128  # 16 key chunks
ROUNDS = [(2 * i, 2) for i in range(8)]  # (start chunk, len)

_PROFILE = False          # test.py flips this for NTFF profiling
_LAST_RESULT = [None]     # stash of the last BassKernelResults (for test.py)
_NC_CACHE = [None]


def _build_nc():
    nc = bacc.Bacc("TRN2", target_bir_lowering=False, debug=False,
                   enable_asserts=True, num_devices=NCORES)

    dt_in = [
        ("xq", [D, NCH], BF),          # init_query chunk (natural chan order)
        ("key", [D, N], BF),           # key_t full
        ("val", [D, N], BF),           # value full
        ("maskT", [N, NCH], BF),       # mask[b,0,n0:n0+NCH,:].T  (m, n)
        ("wqT", [D, D], BF),           # (Wq[ch]/8).T
        ("wkT", [D, D], BF),           # Wk[ch].T
        ("wvT", [D, 520], BF),         # Wv aug with zero cols at ones slots
        ("wmT", [D, D], BF),           # Wm[:, ch].T
        ("w1T", [2 * D, 2 * D], BF),   # (s*W1).T
        ("w2T", [2 * D, D], BF),       # W2.T
        ("bq8", [128, 4], F32),
        ("bk2", [128, 4], F32),
        ("b1pp", [128, 8], F32),
        ("b2v", [128, 4], F32),
    ]
    dr = {}
    for name, shape, dt in dt_in:
        dr[name] = nc.dram_tensor(name, shape, dt, kind="ExternalInput")
    out_d = nc.dram_tensor("out", [D, NCH], F32, kind="ExternalOutput")

    AF = mybir.ActivationFunctionType
    OP = mybir.AluOpType

    with tile.TileContext(nc) as tc:
        with tc.tile_pool(name="const", bufs=1) as cpool:
            # ---------- persistent SBUF tensors; DMAs in consumption order --
            wq_sb = cpool.tile([128, 4, D], BF, tag="wq")
            nc.sync.dma_start(wq_sb[:], dr["wqT"].ap().rearrange("(k p) m -> p k m", p=128))
            xq_sb = cpool.tile([128, 4, NCH], BF, tag="xq")
            nc.sync.dma_start(xq_sb[:], dr["xq"].ap().rearrange("(t p) n -> p t n", p=128))
            wk_sb = cpool.tile([128, 4, D], BF, tag="wk")
            nc.sync.dma_start(wk_sb[:], dr["wkT"].ap().rearrange("(k p) m -> p k m", p=128))
            key_sb = cpool.tile([128, 4, N], BF, tag="key")
            nc.sync.dma_start(key_sb[:], dr["key"].ap().rearrange("(t p) n -> p t n", p=128))
            wv_sb = cpool.tile([128, 4, 520], BF, tag="wv")
            nc.sync.dma_start(wv_sb[:], dr["wvT"].ap().rearrange("(k p) m -> p k m", p=128))
            val_sb = cpool.tile([128, 4, MC, 128], BF, tag="val")
            nc.sync.dma_start(
                val_sb[:],
                dr["val"].ap().rearrange("(t p) (c q) -> p t c q", p=128, q=128))
            bq_sb = cpool.tile([128, 4], F32, tag="bq")
            nc.sync.dma_start(bq_sb[:], dr["bq8"].ap())
            bk_sb = cpool.tile([128, 4], F32, tag="bk")
            nc.sync.dma_start(bk_sb[:], dr["bk2"].ap())
            msk_sb = cpool.tile([128, MC, NCH], BF, tag="mask")
            nc.sync.dma_start(msk_sb[:], dr["maskT"].ap().rearrange("(c p) n -> p c n", p=128))
            b1_sb = cpool.tile([128, 8], F32, tag="b1")
            nc.sync.dma_start(b1_sb[:], dr["b1pp"].ap())
            b2_sb = cpool.tile([128, 4], F32, tag="b2")
            nc.sync.dma_start(b2_sb[:], dr["b2v"].ap())
            # epilogue weights: DMAs emitted late (only needed at ~2/3 of kernel)
            wm_sb = cpool.tile([128, 4, D], BF, tag="wm")
            w1_sb = cpool.tile([128, 8, 2 * D], BF, tag="w1")
            w2_sb = cpool.tile([128, 8, D], BF, tag="w2")

            ones_f = cpool.tile([1, 64], F32, tag="ones_f")
            nc.vector.memset(ones_f[:], 1.0)

            # dummy exp: pulls the ACT table load to kernel start
            dumm = cpool.tile([1, 16], F32, tag="dumm")
            nc.vector.memset(dumm[:], 0.0)
            dumo = cpool.tile([1, 16], BF, tag="dumo")
            nc.scalar.activation(dumo[:], dumm[:], AF.Exp)

            q_sb = cpool.tile([128, 4, NCH], BF, tag="q")
            k_sb = cpool.tile([128, 4, N], BF, tag="k")
            vt_sb = cpool.tile([128, MC, 520], BF, tag="vt")
            xfull_sb = cpool.tile([128, 4, NCH], BF, tag="xfull")
            y_sb = cpool.tile([128, 4, NCH], BF, tag="y")
            h_sb = cpool.tile([128, 8, NCH], BF, tag="h")
            out_sb = cpool.tile([128, 4, NCH], F32, tag="outsb")

            # ---------------- projections ----------------------------------
            with tc.tile_pool(name="pproj", bufs=4, space=bass.MemorySpace.PSUM) as pj:
                # Q projection first (smallest DMA footprint: wq + xq)
                for mo in range(4):
                    ps = pj.tile([128, NCH], F32, tag="pk")
                    for kk in range(4):
                        nc.tensor.matmul(ps[:], wq_sb[:, kk, 128 * mo:128 * mo + 128],
                                         xq_sb[:, kk, :], start=(kk == 0), stop=(kk == 3))
                    nc.any.tensor_scalar_add(q_sb[:, mo, :], ps[:], bq_sb[:, mo:mo + 1])
                # K projection (full sequence)
                for mo in range(4):
                    for mt in range(4):
                        ps = pj.tile([128, 512], F32, tag="pk")
                        for kk in range(4):
                            nc.tensor.matmul(
                                ps[:], wk_sb[:, kk, 128 * mo:128 * mo + 128],
                                key_sb[:, kk, 512 * mt:512 * mt + 512],
                                start=(kk == 0), stop=(kk == 3))
                        nc.any.tensor_scalar_add(
                            k_sb[:, mo, 512 * mt:512 * mt + 512], ps[:], bk_sb[:, mo:mo + 1])
                # V^T projection: vt[m, d'] (520 = 4 pairs x 130, ones cols memset)
                for c in range(MC):
                    for hf in range(2):
                        ps = pj.tile([128, 260], F32, tag="pv")
                        sl = slice(260 * hf, 260 * hf + 260)
                        for kk in range(4):
                            nc.tensor.matmul(ps[:], val_sb[:, kk, c, :], wv_sb[:, kk, sl],
                                             start=(kk == 0), stop=(kk == 3))
                        nc.any.tensor_copy(vt_sb[:, c, sl], ps[:])
                # ones columns at 64 + 65k (k=0..7) in each chunk block
                nc.vector.memset(vt_sb[:, :, 64:520:65], 1.0)

            # ---------------- attention ------------------------------------
            with tc.tile_pool(name="attx", bufs=1) as axp:
                xun_sb = axp.tile([65, 8, NCH], F32, tag="xun")
                rec1_sb = axp.tile([1, 8, NCH], F32, tag="rec1")
                with (
                    tc.tile_pool(name="ptiles", bufs=2) as ppool,
                    tc.tile_pool(name="patt", bufs=2, space=bass.MemorySpace.PSUM) as pa,
                ):
                    def emit_pv(p, pvA, pvB, c0, ln, pmA, pmB):
                        for ci in range(ln):
                            c = c0 + ci
                            nc.tensor.matmul(pvA[:],
                                             vt_sb[:, c, 130 * p:130 * p + 65],
                                             pmA[:, ci, :],
                                             start=(c == 0), stop=(c == MC - 1),
                                             skip_group_check=True)
                            nc.tensor.matmul(pvB[:],
                                             vt_sb[:, c, 130 * p + 65:130 * p + 130],
                                             pmB[:, ci, :],
                                             start=(c == 0), stop=(c == MC - 1),
                                             skip_group_check=True)

                    for p in range(4):  # head pair (2p, 2p+1)
                        pvA = pa.tile([65, NCH], F32, tag="pva")
                        pvB = pa.tile([65, NCH], F32, tag="pvb")
                        pending = None  # software pipeline: PV trails by 1 round
                        for ri, (c0, ln) in enumerate(ROUNDS):
                            scA = pa.tile([128, 2, NCH], F32, tag="sc")
                            scB = pa.tile([128, 2, NCH], F32, tag="sc")
                            for ci in range(ln):
                                c = c0 + ci
                                nc.tensor.matmul(scA[:, ci, :],
                                                 k_sb[0:64, p, 128 * c:128 * c + 128],
                                                 q_sb[0:64, p, :], start=True, stop=True)
                                nc.tensor.matmul(scB[:, ci, :],
                                                 k_sb[64:128, p, 128 * c:128 * c + 128],
                                                 q_sb[64:128, p, :], start=True, stop=True)
                            if pending is not None:
                                emit_pv(p, pvA, pvB, *pending)
                            pA = ppool.tile([128, 2, NCH], BF, tag="pa")
                            pB = ppool.tile([128, 2, NCH], BF, tag="pb")
                            nc.scalar.activation(pA[:, 0:ln, :], scA[:, 0:ln, :], AF.Exp)
                            nc.scalar.activation(pB[:, 0:ln, :], scB[:, 0:ln, :], AF.Exp)
                            pmA = ppool.tile([128, 2, NCH], BF, tag="pma")
                            pmB = ppool.tile([128, 2, NCH], BF, tag="pmb")
                            nc.vector.tensor_tensor(pmA[:, 0:ln, :], pA[:, 0:ln, :],
                                                    msk_sb[:, c0:c0 + ln, :], OP.mult)
                            # balance the mask multiplies: gpsimd is ~2x slower
                            engB = nc.gpsimd if (ri % 2 == 0) else nc.vector
                            engB.tensor_tensor(pmB[:, 0:ln, :], pB[:, 0:ln, :],
                                               msk_sb[:, c0:c0 + ln, :], OP.mult)
                            pending = (c0, ln, pmA, pmB)
                        emit_pv(p, pvA, pvB, *pending)
                        nc.any.tensor_copy(xun_sb[:, 2 * p, :], pvA[:])
                        nc.any.tensor_copy(xun_sb[:, 2 * p + 1, :], pvB[:])
                        # per-pair denominator pipeline (overlaps next pair)
                        den_p = ppool.tile([2, NCH], F32, tag="denp", name=f"den{p}", bufs=1)
                        nc.sync.dma_start(den_p[:], xun_sb[64:65, 2 * p:2 * p + 2, :])
                        lnd_p = ppool.tile([2, NCH], F32, tag="lndp", name=f"lnd{p}", bufs=1)
                        nc.scalar.activation(lnd_p[:], den_p[:], AF.Ln)
                        # rec = exp(-ln(den)): stays in the natural_log_exp set
                        rec_p = ppool.tile([2, NCH], F32, tag="recp", name=f"rec{p}", bufs=1)
                        nc.scalar.activation(rec_p[:], lnd_p[:], AF.Exp, scale=-1.0)
                        nc.sync.dma_start(rec1_sb[0:1, 2 * p:2 * p + 2, :], rec_p[:])

                # late epilogue-weight DMAs (overlap attention)
                nc.sync.dma_start(wm_sb[:], dr["wmT"].ap().rearrange("(k p) m -> p k m", p=128))
                nc.sync.dma_start(w1_sb[:], dr["w1T"].ap().rearrange("(k p) m -> p k m", p=128))
                nc.sync.dma_start(w2_sb[:], dr["w2T"].ap().rearrange("(k p) m -> p k m", p=128))

                # ------------- normalization + epilogue --------------------
                with (
                    tc.tile_pool(name="nrm", bufs=1) as nrp,
                    tc.tile_pool(name="pep", bufs=1, space=bass.MemorySpace.PSUM) as pep,
                ):
                    xnorm_sb = nrp.tile([64, 8, NCH], BF, tag="xnorm")

                    # W1-x preheat: x-half of W1 runs while normalization drains
                    hx = []
                    for mo in range(6):
                        ps = pep.tile([128, NCH], F32, tag=f"hx{mo}", name=f"hxt{mo}")
                        hx.append(ps)
                        for kk in range(4, 8):
                            nc.tensor.matmul(ps[:], w1_sb[:, kk, 128 * mo:128 * mo + 128],
                                             xq_sb[:, kk - 4, :], start=(kk == 4),
                                             stop=False, skip_group_check=True)

                    bcs = [pep.tile([64, NCH], F32, tag=f"bc{i}", name=f"bct{i}")
                           for i in range(2)]
                    for hh in range(8):
                        bc = bcs[hh % 2]
                        nc.tensor.matmul(bc[:], ones_f[0:1, :], rec1_sb[0:1, hh, :],
                                         start=True, stop=True)
                        nc.any.tensor_tensor(xnorm_sb[:, hh, :], xun_sb[0:64, hh, :],
                                             bc[:], OP.mult)
                    # partition shift: even heads -> partitions 0-63; odd -> 64-127
                    nc.sync.dma_start(xfull_sb[0:64, :, :], xnorm_sb[:, 0:8:2, :])
                    nc.sync.dma_start(xfull_sb[64:128, :, :], xnorm_sb[:, 1:8:2, :])

                    # Wm (uses the 2 bc slots' banks via separate tags is fine:
                    # bc tiles are released after the last tensor_tensor)
                    for mo in range(4):
                        ps = pep.tile([128, NCH], F32, tag=f"bc{mo % 2}")
                        for kk in range(4):
                            nc.tensor.matmul(ps[:], wm_sb[:, kk, 128 * mo:128 * mo + 128],
                                             xfull_sb[:, kk, :], start=(kk == 0),
                                             stop=(kk == 3))
                        nc.any.tensor_copy(y_sb[:, mo, :], ps[:])
                    # W1 y-half accumulates onto the preheated psum tiles
                    for mo in range(8):
                        if mo < 6:
                            ps = hx[mo]
                            for kk in range(4):
                                nc.tensor.matmul(ps[:], w1_sb[:, kk, 128 * mo:128 * mo + 128],
                                                 y_sb[:, kk, :], start=False,
                                                 stop=(kk == 3), skip_group_check=True)
                        else:
                            ps = pep.tile([128, NCH], F32, tag=f"bc{mo % 2}")
                            for kk in range(8):
                                rhs = y_sb[:, kk, :] if kk < 4 else xq_sb[:, kk - 4, :]
                                nc.tensor.matmul(ps[:], w1_sb[:, kk, 128 * mo:128 * mo + 128],
                                                 rhs, start=(kk == 0), stop=(kk == 7))
                        # relu(x + b): (ps + b1pp) max 0
                        nc.any.tensor_scalar(h_sb[:, mo, :], ps[:], b1_sb[:, mo:mo + 1],
                                             0.0, OP.add, OP.max)
                    for mo in range(4):
                        ps = pep.tile([128, NCH], F32, tag=f"hx{mo}")
                        for kk in range(8):
                            nc.tensor.matmul(ps[:], w2_sb[:, kk, 128 * mo:128 * mo + 128],
                                             h_sb[:, kk, :], start=(kk == 0), stop=(kk == 7))
                        nc.any.tensor_scalar_add(out_sb[:, mo, :], ps[:], b2_sb[:, mo:mo + 1])
                        nc.sync.dma_start(
                            out_d.ap().rearrange("(t p) n -> p t n", p=128)[:, mo, :],
                            out_sb[:, mo, :])

    nc.compile()
    return nc


def _prep_inputs(inputs):
    """Host-side slicing / casting / weight folding -> per-core input maps."""
    f = {k: np.asarray(v, np.float32) if np.asarray(v).dtype != np.int32
         else np.asarray(v) for k, v in inputs.items()}
    ch = np.array([dk * H + h for h in range(H) for dk in range(DK)])

    s = f["gamma"] / np.sqrt(f["rvar"] + EPS)
    W1p = s[:, None] * f["W1"]
    bmp = f["Wm"] @ f["bv"] + f["bm"]
    b1pp = W1p[:, :D] @ bmp + s * (f["b1"] - f["rmean"]) + f["beta"]

    wqT = np.ascontiguousarray((f["Wq"][ch] / 8).T).astype(BF16)
    wkT = np.ascontiguousarray(f["Wk"][ch].T).astype(BF16)
    wvT = np.zeros((D, 520), np.float32)
    for p in range(4):
        wvT[:, 130 * p:130 * p + 64] = f["Wv"][ch[128 * p:128 * p + 64]].T
        wvT[:, 130 * p + 65:130 * p + 129] = f["Wv"][ch[128 * p + 64:128 * p + 128]].T
    wmT = np.ascontiguousarray(f["Wm"][:, ch].T).astype(BF16)
    w1T = np.ascontiguousarray(W1p.T).astype(BF16)
    w2T = np.ascontiguousarray(f["W2"].T).astype(BF16)

    shared = {
        "wqT": wqT, "wkT": wkT, "wvT": wvT.astype(BF16),
        "wmT": wmT, "w1T": w1T, "w2T": w2T,
        "bq8": np.ascontiguousarray((f["bq"][ch] / 8).reshape(4, 128).T).astype(np.float32),
        "bk2": np.ascontiguousarray(f["bk"][ch].reshape(4, 128).T).astype(np.float32),
        "b1pp": np.ascontiguousarray(b1pp.reshape(8, 128).T).astype(np.float32),
        "b2v": np.ascontiguousarray(f["b2"].reshape(4, 128).T).astype(np.float32),
    }
    in_maps = []
    for c in range(NCORES):
        b, j = divmod(c, G)
        n0 = j * NCH
        m = dict(shared)
        m["xq"] = np.ascontiguousarray(f["init_query"][b][:, n0:n0 + NCH]).astype(BF16)
        m["key"] = f["key_t"][b].astype(BF16)
        m["val"] = f["value"][b].astype(BF16)
        m["maskT"] = np.ascontiguousarray(f["mask"][b, 0, n0:n0 + NCH, :].T).astype(BF16)
        in_maps.append(m)
    return in_maps


def kernel(**inputs) -> np.ndarray:
    if _NC_CACHE[0] is None:
        _NC_CACHE[0] = _build_nc()
    nc = _NC_CACHE[0]
    in_maps = _prep_inputs(inputs)
    res = run_bass_kernel_spmd(nc, in_maps, list(range(NCORES)), trace=_PROFILE)
    _LAST_RESULT[0] = res
    out = np.zeros((B, D, N), np.float32)
    for c in range(NCORES):
        b, j = divmod(c, G)
        out[b, :, j * NCH:(j + 1) * NCH] = res.results[c]["out"]
    return out


# revision 22
# speedup vs baseline: 1.2687x; 1.1562x over previous
"""Trainium2 Bass kernel for nn_MultiHeadAttention_51711406244477.

Sharding: 8 cores = (batch b in 0..1) x (sequence chunk j in 0..3).
Each core computes the full module output for its (batch, 512-position chunk):
  - K/V projections over the full sequence (replicated within a batch group)
  - Q projection for its chunk only
  - attention (all 8 heads) for its 512 query positions
  - Wm -> concat -> W1(+folded BN)+ReLU -> W2 epilogue for its chunk
No collectives; host slices/casts inputs per core and reassembles the output.

Precision: bf16 activations/weights on the PE, fp32 PSUM accumulation,
fp32 softmax denominators. Softmax is computed without max-subtraction
(scores are bounded, exp cannot overflow fp32) and masking is applied
multiplicatively after exp, which matches where(mask==0, -1e9) exactly
because masked exp terms underflow to 0 in the reference too.

All matmuls are built as full-array (K=128 contraction, M=128 output
partitions) ops: the PE HAM clock gate only unthrottles to 2.4 GHz under
high array activity, and half-array matmuls (K=64 heads / M=65 PV) were
measured to keep the clock at 1.2 GHz for the whole attention phase.
  - scores: stationary k is the full 128-row chunk (both heads of a pair);
    the moving q is zero-padded per head (qA top-half live, qB bottom-half
    live), so each score matmul contracts over 128 rows but only one
    head's 64 contribute.
  - P@V: stationary is [v_head(64) | ones(64)] -> M=128; output rows 0-63
    are the attention output, rows 64-127 all hold the softmax denominator.

Host-side foldings:
  - 1/sqrt(dk) into Wq/bq
  - BatchNorm (inference) into W1/b1
  - bv into b1 (via Wm), bm into b1
  - channel permutation to head-major order into Wq/Wk/Wv rows & Wm columns
"""

import numpy as np
import ml_dtypes

import concourse.bacc as bacc
import concourse.bass as bass
import concourse.mybir as mybir
import concourse.tile as tile
from concourse.bass_utils import run_bass_kernel_spmd

BF16 = ml_dtypes.bfloat16
F32 = mybir.dt.float32
BF = mybir.dt.bfloat16

B, D, N, H, DK = 2, 512, 2048, 8, 64
EPS = 1e-5
G = 4          # chunks (cores) per batch
NCH = N // G   # 512 query positions per core
NCORES = 8
MC = N // 128  # 16 key chunks
ROUNDS = [(2 * i, 2) for i in range(8)]  # (start chunk, len)

_PROFILE = False          # test.py flips this for NTFF profiling
_LAST_RESULT = [None]     # stash of the last BassKernelResults (for test.py)
_NC_CACHE = [None]


def _build_nc():
    nc = bacc.Bacc("TRN2", target_bir_lowering=False, debug=False,
                   enable_asserts=True, num_devices=NCORES)

    dt_in = [
        ("xq", [D, NCH], BF),          # init_query chunk (natural chan order)
        ("key", [D, N], BF),           # key_t full
        ("val", [D, N], BF),           # value full
        ("maskT", [N, NCH], BF),       # mask[b,0,n0:n0+NCH,:].T  (m, n)
        ("wqT", [D, D], BF),           # (Wq[ch]/8).T
        ("wkT", [D, D], BF),           # Wk[ch].T
        ("wvT", [D, D], BF),           # Wv[ch].T
        ("wmT", [D, D], BF),           # Wm[:, ch].T
        ("w1T", [2 * D, 2 * D], BF),   # (s*W1).T
        ("w2T", [2 * D, D], BF),       # W2.T
        ("bq8", [128, 4], F32),
        ("bk2", [128, 4], F32),
        ("b1pp", [128, 8], F32),
        ("b2v", [128, 4], F32),
    ]
    dr = {}
    for name, shape, dt in dt_in:
        dr[name] = nc.dram_tensor(name, shape, dt, kind="ExternalInput")
    out_d = nc.dram_tensor("out", [D, NCH], F32, kind="ExternalOutput")

    AF = mybir.ActivationFunctionType
    OP = mybir.AluOpType

    with tile.TileContext(nc) as tc:
        with tc.tile_pool(name="const", bufs=1) as cpool:
            # ---------- persistent SBUF tensors; DMAs in consumption order --
            wq_sb = cpool.tile([128, 4, D], BF, tag="wq")
            nc.sync.dma_start(wq_sb[:], dr["wqT"].ap().rearrange("(k p) m -> p k m", p=128))
            xq_sb = cpool.tile([128, 4, NCH], BF, tag="xq")
            nc.sync.dma_start(xq_sb[:], dr["xq"].ap().rearrange("(t p) n -> p t n", p=128))
            wk_sb = cpool.tile([128, 4, D], BF, tag="wk")
            nc.sync.dma_start(wk_sb[:], dr["wkT"].ap().rearrange("(k p) m -> p k m", p=128))
            wv_sb = cpool.tile([128, 4, D], BF, tag="wv")
            nc.sync.dma_start(wv_sb[:], dr["wvT"].ap().rearrange("(k p) m -> p k m", p=128))
            bq_sb = cpool.tile([128, 4], F32, tag="bq")
            nc.sync.dma_start(bq_sb[:], dr["bq8"].ap())
            bk_sb = cpool.tile([128, 4], F32, tag="bk")
            nc.sync.dma_start(bk_sb[:], dr["bk2"].ap())
            b1_sb = cpool.tile([128, 8], F32, tag="b1")
            nc.sync.dma_start(b1_sb[:], dr["b1pp"].ap())
            b2_sb = cpool.tile([128, 4], F32, tag="b2")
            nc.sync.dma_start(b2_sb[:], dr["b2v"].ap())
            msk_sb = cpool.tile([128, MC, NCH], BF, tag="mask")
            nc.sync.dma_start(msk_sb[:], dr["maskT"].ap().rearrange("(c p) n -> p c n", p=128))
            # epilogue weights: DMAs emitted late (only needed at ~2/3 of kernel)
            wm_sb = cpool.tile([128, 4, D], BF, tag="wm")
            w1_sb = cpool.tile([128, 8, 2 * D], BF, tag="w1")
            w2_sb = cpool.tile([128, 8, D], BF, tag="w2")

            ones_f = cpool.tile([1, 64], F32, tag="ones_f")
            nc.vector.memset(ones_f[:], 1.0)

            # dummy exp: pulls the ACT table load to kernel start
            dumm = cpool.tile([1, 16], F32, tag="dumm")
            nc.vector.memset(dumm[:], 0.0)
            dumo = cpool.tile([1, 16], BF, tag="dumo")
            nc.scalar.activation(dumo[:], dumm[:], AF.Exp)

            # zero-padded per-head q: qA has head 2p in rows 0-63, zeros below;
            # qB has head 2p+1 in rows 64-127, zeros above.
            qA_sb = cpool.tile([128, 4, NCH], BF, tag="qA")
            nc.vector.memset(qA_sb[64:128, :, :], 0.0)
            qB_sb = cpool.tile([128, 4, NCH], BF, tag="qB")
            nc.vector.memset(qB_sb[0:64, :, :], 0.0)
            k_sb = cpool.tile([128, 4, N], BF, tag="k")
            # vt layout per pair p (256 cols): [vA(64) | ones | vB(64) | ones]
            # vt layout per pair p (192 cols): [vA(64) | ones(64) | vB(64)];
            # lhsT A = cols 0..128 ([vA|ones]), lhsT B = cols 64..192 ([ones|vB])
            vt_sb = cpool.tile([128, MC, 768], BF, tag="vt")
            nc.vector.memset(
                vt_sb[:].rearrange("p c (pr x) -> p c pr x", x=192)[:, :, :, 64:128], 1.0)
            xfull_sb = cpool.tile([128, 4, NCH], BF, tag="xfull")
            y_sb = cpool.tile([128, 4, NCH], BF, tag="y")
            h_sb = cpool.tile([128, 8, NCH], BF, tag="h")
            out_sb = cpool.tile([128, 4, NCH], F32, tag="outsb")

            # ---------------- projections ----------------------------------
            with (
                tc.tile_pool(name="kvin", bufs=1) as kvp,
                tc.tile_pool(name="pproj", bufs=4, space=bass.MemorySpace.PSUM) as pj,
            ):
                key_sb = kvp.tile([128, 4, N], BF, tag="key")
                nc.sync.dma_start(key_sb[:],
                                  dr["key"].ap().rearrange("(t p) n -> p t n", p=128))
                val_sb = kvp.tile([128, 4, MC, 128], BF, tag="val")
                nc.sync.dma_start(
                    val_sb[:],
                    dr["val"].ap().rearrange("(t p) (c q) -> p t c q", p=128, q=128))
                # Q projection first (smallest DMA footprint: wq + xq)
                for mo in range(4):
                    ps = pj.tile([128, NCH], F32, tag="pk")
                    for kk in range(4):
                        nc.tensor.matmul(ps[:], wq_sb[:, kk, 128 * mo:128 * mo + 128],
                                         xq_sb[:, kk, :], start=(kk == 0), stop=(kk == 3))
                    nc.any.tensor_scalar_add(qA_sb[0:64, mo, :], ps[0:64, :],
                                             bq_sb[0:64, mo:mo + 1])
                    nc.any.tensor_scalar_add(qB_sb[64:128, mo, :], ps[64:128, :],
                                             bq_sb[64:128, mo:mo + 1])
                # K projection (full sequence)
                for mo in range(4):
                    for mt in range(4):
                        ps = pj.tile([128, 512], F32, tag="pk")
                        for kk in range(4):
                            nc.tensor.matmul(
                                ps[:], wk_sb[:, kk, 128 * mo:128 * mo + 128],
                                key_sb[:, kk, 512 * mt:512 * mt + 512],
                                start=(kk == 0), stop=(kk == 3))
                        nc.any.tensor_scalar_add(
                            k_sb[:, mo, 512 * mt:512 * mt + 512], ps[:], bk_sb[:, mo:mo + 1])
                # V^T projection: psum cols are head-major [A0 B0 A1 B1 ...];
                # scatter head g to vt cols 128g..128g+64
                for c in range(MC):
                    ps = pj.tile([128, D], F32, tag="pv")
                    for kk in range(4):
                        nc.tensor.matmul(ps[:], val_sb[:, kk, c, :], wv_sb[:, kk, :],
                                         start=(kk == 0), stop=(kk == 3))
                    vtv = vt_sb[:, c, :].rearrange("p (pr x) -> p pr x", x=192)
                    psv = ps[:].rearrange("p (g i) -> p g i", i=64)
                    nc.any.tensor_copy(vtv[:, :, 0:64], psv[:, 0::2, :])
                    nc.any.tensor_copy(vtv[:, :, 128:192], psv[:, 1::2, :])

            # ---------------- attention ------------------------------------
            with tc.tile_pool(name="attx", bufs=1) as axp:
                # X copies: even heads live at partitions 0-63, odd at 64-127
                xeven_sb = axp.tile([64, 4, NCH], BF, tag="xeven")
                xodd_sb = axp.tile([128, 4, NCH], BF, tag="xodd")
                # denominator->reciprocal holders (in-place Ln then Exp)
                dre_sb = axp.tile([65, 4, NCH], F32, tag="dre")   # evens: row 64
                dro_sb = axp.tile([1, 4, NCH], F32, tag="dro")    # odds: row 0
                ones64_sb = axp.tile([65, 64], F32, tag="ones64")
                nc.vector.memset(ones64_sb[64:65, :], 1.0)
                with (
                    tc.tile_pool(name="ptiles", bufs=2) as ppool,
                    tc.tile_pool(name="patt", bufs=2, space=bass.MemorySpace.PSUM) as pa,
                ):
                    def emit_pv(p, pvA, pvB, c0, ln, pmA, pmB):
                        for ci in range(ln):
                            c = c0 + ci
                            nc.tensor.matmul(pvA[:],
                                             vt_sb[:, c, 192 * p:192 * p + 128],
                                             pmA[:, ci, :],
                                             start=(c == 0), stop=(c == MC - 1),
                                             skip_group_check=True)
                            nc.tensor.matmul(pvB[:],
                                             vt_sb[:, c, 192 * p + 64:192 * p + 192],
                                             pmB[:, ci, :],
                                             start=(c == 0), stop=(c == MC - 1),
                                             skip_group_check=True)

                    for p in range(4):  # head pair (2p, 2p+1)
                        pvA = pa.tile([128, NCH], F32, tag="pva")
                        pvB = pa.tile([128, NCH], F32, tag="pvb")
                        pending = None  # software pipeline: PV trails by 1 round
                        for ri, (c0, ln) in enumerate(ROUNDS):
                            scA = pa.tile([128, 2, NCH], F32, tag="sc")
                            scB = pa.tile([128, 2, NCH], F32, tag="sc")
                            for ci in range(ln):
                                c = c0 + ci
                                nc.tensor.matmul(scA[:, ci, :],
                                                 k_sb[:, p, 128 * c:128 * c + 128],
                                                 qA_sb[:, p, :], start=True, stop=True)
                                nc.tensor.matmul(scB[:, ci, :],
                                                 k_sb[:, p, 128 * c:128 * c + 128],
                                                 qB_sb[:, p, :], start=True, stop=True)
                            if pending is not None:
                                emit_pv(p, pvA, pvB, *pending)
                            pA = ppool.tile([128, 2, NCH], BF, tag="pa")
                            pB = ppool.tile([128, 2, NCH], BF, tag="pb")
                            nc.scalar.activation(pA[:, 0:ln, :], scA[:, 0:ln, :], AF.Exp)
                            nc.scalar.activation(pB[:, 0:ln, :], scB[:, 0:ln, :], AF.Exp)
                            pmA = ppool.tile([128, 2, NCH], BF, tag="pma")
                            pmB = ppool.tile([128, 2, NCH], BF, tag="pmb")
                            nc.vector.tensor_tensor(pmA[:, 0:ln, :], pA[:, 0:ln, :],
                                                    msk_sb[:, c0:c0 + ln, :], OP.mult)
                            # balance the mask multiplies: gpsimd is ~2x slower
                            engB = nc.gpsimd if (ri % 2 == 0) else nc.vector
                            engB.tensor_tensor(pmB[:, 0:ln, :], pB[:, 0:ln, :],
                                               msk_sb[:, c0:c0 + ln, :], OP.mult)
                            pending = (c0, ln, pmA, pmB)
                        emit_pv(p, pvA, pvB, *pending)
                        # X: evens rows 0-63 of pvA; odds rows 64-127 of pvB
                        nc.any.tensor_copy(xeven_sb[:, p, :], pvA[0:64, :])
                        nc.any.tensor_copy(xodd_sb[64:128, p, :], pvB[64:128, :])
                        # denominators: pvA row 64 (evens), pvB row 0 (odds);
                        # then rec = exp(-ln(den)) in place (same table set)
                        nc.any.tensor_copy(dre_sb[64:65, p, :], pvA[64:65, :])
                        nc.any.tensor_copy(dro_sb[0:1, p, :], pvB[0:1, :])
                        nc.scalar.activation(dre_sb[64:65, p, :], dre_sb[64:65, p, :], AF.Ln)
                        nc.scalar.activation(dre_sb[64:65, p, :], dre_sb[64:65, p, :],
                                             AF.Exp, scale=-1.0)
                        nc.scalar.activation(dro_sb[0:1, p, :], dro_sb[0:1, p, :], AF.Ln)
                        nc.scalar.activation(dro_sb[0:1, p, :], dro_sb[0:1, p, :],
                                             AF.Exp, scale=-1.0)

                # late epilogue-weight DMAs (overlap attention)
                nc.sync.dma_start(wm_sb[:], dr["wmT"].ap().rearrange("(k p) m -> p k m", p=128))
                nc.sync.dma_start(w1_sb[:], dr["w1T"].ap().rearrange("(k p) m -> p k m", p=128))
                nc.sync.dma_start(w2_sb[:], dr["w2T"].ap().rearrange("(k p) m -> p k m", p=128))

                # ------------- normalization + epilogue --------------------
                with (
                    tc.tile_pool(name="nrm", bufs=1) as nrp,
                    tc.tile_pool(name="pep", bufs=1, space=bass.MemorySpace.PSUM) as pep,
                ):
                    # W1-x preheat: x-half of W1 runs while normalization drains
                    hx = []
                    for mo in range(6):
                        ps = pep.tile([128, NCH], F32, tag=f"hx{mo}", name=f"hxt{mo}")
                        hx.append(ps)
                        for kk in range(4, 8):
                            nc.tensor.matmul(ps[:], w1_sb[:, kk, 128 * mo:128 * mo + 128],
                                             xq_sb[:, kk - 4, :], start=(kk == 4),
                                             stop=False, skip_group_check=True)

                    bcs = [pep.tile([128, NCH], F32, tag=f"bc{i}", name=f"bct{i}")
                           for i in range(2)]
                    for p in range(4):
                        bc = bcs[p % 2]
                        # broadcast 1/den across partitions via rank-1 matmuls
                        nc.tensor.matmul(bc[0:64, :], ones64_sb[64:65, :],
                                         dre_sb[64:65, p, :], start=True, stop=True)
                        nc.tensor.matmul(bc[64:128, :], ones_f[0:1, :],
                                         dro_sb[0:1, p, :], start=True, stop=True,
                                         tile_position=(0, 64))
                        nc.any.tensor_tensor(xfull_sb[0:64, p, :], xeven_sb[:, p, :],
                                             bc[0:64, :], OP.mult)
                        nc.any.tensor_tensor(xfull_sb[64:128, p, :],
                                             xodd_sb[64:128, p, :],
                                             bc[64:128, :], OP.mult)

                    # Wm
                    for mo in range(4):
                        ps = pep.tile([128, NCH], F32, tag=f"bc{mo % 2}", name=f"wmps{mo}")
                        for kk in range(4):
                            nc.tensor.matmul(ps[:], wm_sb[:, kk, 128 * mo:128 * mo + 128],
                                             xfull_sb[:, kk, :], start=(kk == 0),
                                             stop=(kk == 3))
                        nc.any.tensor_copy(y_sb[:, mo, :], ps[:])
                    # W1 y-half accumulates onto the preheated psum tiles
                    for mo in range(8):
                        if mo < 6:
                            ps = hx[mo]
                            for kk in range(4):
                                nc.tensor.matmul(ps[:], w1_sb[:, kk, 128 * mo:128 * mo + 128],
                                                 y_sb[:, kk, :], start=False,
                                                 stop=(kk == 3), skip_group_check=True)
                        else:
                            ps = pep.tile([128, NCH], F32, tag=f"bc{mo % 2}",
                                          name=f"w1ps{mo}")
                            for kk in range(8):
                                rhs = y_sb[:, kk, :] if kk < 4 else xq_sb[:, kk - 4, :]
                                nc.tensor.matmul(ps[:], w1_sb[:, kk, 128 * mo:128 * mo + 128],
                                                 rhs, start=(kk == 0), stop=(kk == 7))
                        # relu(x + b): (ps + b1pp) max 0
                        nc.any.tensor_scalar(h_sb[:, mo, :], ps[:], b1_sb[:, mo:mo + 1],
                                             0.0, OP.add, OP.max)
                    for mo in range(4):
                        ps = pep.tile([128, NCH], F32, tag=f"hx{mo}", name=f"w2ps{mo}")
                        for kk in range(8):
                            nc.tensor.matmul(ps[:], w2_sb[:, kk, 128 * mo:128 * mo + 128],
                                             h_sb[:, kk, :], start=(kk == 0), stop=(kk == 7))
                        nc.any.tensor_scalar_add(out_sb[:, mo, :], ps[:], b2_sb[:, mo:mo + 1])
                        nc.sync.dma_start(
                            out_d.ap().rearrange("(t p) n -> p t n", p=128)[:, mo, :],
                            out_sb[:, mo, :])

    nc.compile()
    return nc


def _prep_inputs(inputs):
    """Host-side slicing / casting / weight folding -> per-core input maps."""
    f = {k: np.asarray(v, np.float32) if np.asarray(v).dtype != np.int32
         else np.asarray(v) for k, v in inputs.items()}
    ch = np.array([dk * H + h for h in range(H) for dk in range(DK)])

    s = f["gamma"] / np.sqrt(f["rvar"] + EPS)
    W1p = s[:, None] * f["W1"]
    bmp = f["Wm"] @ f["bv"] + f["bm"]
    b1pp = W1p[:, :D] @ bmp + s * (f["b1"] - f["rmean"]) + f["beta"]

    shared = {
        "wqT": np.ascontiguousarray((f["Wq"][ch] / 8).T).astype(BF16),
        "wkT": np.ascontiguousarray(f["Wk"][ch].T).astype(BF16),
        "wvT": np.ascontiguousarray(f["Wv"][ch].T).astype(BF16),
        "wmT": np.ascontiguousarray(f["Wm"][:, ch].T).astype(BF16),
        "w1T": np.ascontiguousarray(W1p.T).astype(BF16),
        "w2T": np.ascontiguousarray(f["W2"].T).astype(BF16),
        "bq8": np.ascontiguousarray((f["bq"][ch] / 8).reshape(4, 128).T).astype(np.float32),
        "bk2": np.ascontiguousarray(f["bk"][ch].reshape(4, 128).T).astype(np.float32),
        "b1pp": np.ascontiguousarray(b1pp.reshape(8, 128).T).astype(np.float32),
        "b2v": np.ascontiguousarray(f["b2"].reshape(4, 128).T).astype(np.float32),
    }
    in_maps = []
    for c in range(NCORES):
        b, j = divmod(c, G)
        n0 = j * NCH
        m = dict(shared)
        m["xq"] = np.ascontiguousarray(f["init_query"][b][:, n0:n0 + NCH]).astype(BF16)
        m["key"] = f["key_t"][b].astype(BF16)
        m["val"] = f["value"][b].astype(BF16)
        m["maskT"] = np.ascontiguousarray(f["mask"][b, 0, n0:n0 + NCH, :].T).astype(BF16)
        in_maps.append(m)
    return in_maps


def kernel(**inputs) -> np.ndarray:
    if _NC_CACHE[0] is None:
        _NC_CACHE[0] = _build_nc()
    nc = _NC_CACHE[0]
    in_maps = _prep_inputs(inputs)
    res = run_bass_kernel_spmd(nc, in_maps, list(range(NCORES)), trace=_PROFILE)
    _LAST_RESULT[0] = res
    out = np.zeros((B, D, N), np.float32)
    for c in range(NCORES):
        b, j = divmod(c, G)
        out[b, :, j * NCH:(j + 1) * NCH] = res.results[c]["out"]
    return out
